# revision 1
# baseline (speedup 1.0000x reference)
"""Multi-head causal attention (B=2, S=2048, D=2048, H=16) on 8 TRN2 NeuronCores.

Sharding (host-side): core c in 0..7 handles batch b=c//4 and heads
4*(c%4)..4*(c%4)+4 (a 512-wide column slice of wq/wk/wv, row slice of wp).
Each core computes its 4 heads' attention and a partial output projection
[S, D]; the host sums the 4 partials per batch and adds bp.

Per-core kernel (all matmuls in float32r -> full PE speed, ~2e-4 rel err):
  A) QKV projections from host-pre-transposed xT (d-major):
       qT/kT per head in [hd=128, S] layout, v per head in [S, hd] natural
       layout, written to DRAM scratch.
  B) Per head, per 512-wide q chunk j: scoresT = K_tile @ Q_chunk in
     [keys, q] layout (causal: only key tiles <= diagonal).  The causal mask
     for diagonal blocks is ADDED IN PSUM by a second matmul
     (identity^T @ mask_slice), keeping DVE off the critical path.  exp via
     ACT with the 1/sqrt(hd) scale folded in.  ctxT[hd, q] and the softmax
     denominators (ones^T @ pT partition sums) accumulate in PSUM over key
     tiles; reciprocal + ones-outer-product broadcast + one DVE multiply
     normalize into ctxT.
  C) Output projection: out[q, :] += ctxT_h^T @ wp_h accumulated over heads.
"""
import sys
if "/opt/trn_rl_repo" not in sys.path:
    sys.path.insert(0, "/opt/trn_rl_repo")

import numpy as np

B, S, D = 2, 2048, 2048
H, HD = 16, 128
NCORES = 8
HH = 4            # heads per core
CW = HH * HD      # 512 column slice per core
P = 128
KT = D // P       # 16 contraction tiles
NQC = 4           # q chunks of 512
NKT = S // P      # 16 key tiles
SCALE = 1.0 / float(np.sqrt(HD))
MASK_NEG = -1.0e10

_cache = {}


def _build():
    import concourse.bass as bass
    import concourse.tile as tile
    from concourse import bacc, mybir

    F32 = mybir.dt.float32
    F32R = mybir.dt.float32r
    AF = mybir.ActivationFunctionType
    ALU = mybir.AluOpType

    nc = bacc.Bacc("TRN2", target_bir_lowering=False, debug=False, num_devices=NCORES)

    xt = nc.dram_tensor("xt", [D, S], F32R, kind="ExternalInput")      # x[b].T
    wq = nc.dram_tensor("wq", [D, CW], F32R, kind="ExternalInput")
    wk = nc.dram_tensor("wk", [D, CW], F32R, kind="ExternalInput")
    wv = nc.dram_tensor("wv", [D, CW], F32R, kind="ExternalInput")
    wp = nc.dram_tensor("wp", [CW, D], F32R, kind="ExternalInput")
    # bq/bk arrive host-pre-arranged as [p, h] so the load is contiguous
    bq = nc.dram_tensor("bq", [P, HH], F32, kind="ExternalInput")
    bk = nc.dram_tensor("bk", [P, HH], F32, kind="ExternalInput")
    bv = nc.dram_tensor("bv", [CW], F32, kind="ExternalInput")
    out = nc.dram_tensor("out", [S, D], F32, kind="ExternalOutput")

    with tile.TileContext(nc) as tc:
        with tc.tile_pool(name="consts", bufs=1) as consts, \
             tc.tile_pool(name="dram", bufs=1, space="DRAM") as dram:
            # DRAM scratch: qT/kT [head][hd, S], v [head][S, hd] (contiguous
            # per-head so phase B loads are linear 64KB copies)
            qT_d = dram.tile([HH, P, S], F32R)
            v_d = dram.tile([S, CW], F32R)

            # per-head per-partition biases for qT/kT layout: [p, h]
            # (SWDGE so the sync/scalar queues start on xt immediately)
            bq_sb = consts.tile([P, HH], F32)
            bk_sb = consts.tile([P, HH], F32)
            bv_sb = consts.tile([P, CW], F32)
            # ones vectors (fp32r) for denominator / broadcast matmuls
            ones_f32 = consts.tile([P, 1], F32)
            nc.vector.memset(ones_f32, 1.0)
            ones_col = consts.tile([P, 1], F32R)
            nc.vector.tensor_copy(ones_col, ones_f32)
            ones_row_f32 = consts.tile([1, P], F32)
            nc.vector.memset(ones_row_f32, 1.0)
            ones_row = consts.tile([1, P], F32R)
            nc.vector.tensor_copy(ones_row, ones_row_f32)
            # causal masks + identity built up front (values 0/-1e10/1 are
            # exact in any float width, so build straight into fp32r)
            mask_r = consts.tile([P, 896], F32R)
            nc.vector.memset(mask_r.bitcast(F32), 0.0)
            nc.gpsimd.affine_select(
                out=mask_r, in_=mask_r,
                compare_op=ALU.is_ge, fill=MASK_NEG,
                base=-384, channel_multiplier=-1, pattern=[[1, 896]],
            )
            ident_r = consts.tile([P, P], F32R)
            nc.vector.memset(ident_r.bitcast(F32), 0.0)
            nc.gpsimd.affine_select(
                out=ident_r, in_=ident_r,
                compare_op=ALU.not_equal, fill=1.0,
                base=0, channel_multiplier=1, pattern=[[-1, P]],
            )

            # kT for all heads stays in SBUF through phase B (saves the
            # DRAM round-trip and B-side reloads)
            kT_all = consts.tile([P, HH, S], F32R)

            # ---------------- Phase A: QKV projections ----------------
            with tc.tile_pool(name="xt_pool", bufs=2 * KT) as xt_pool, \
                 tc.tile_pool(name="w_pool", bufs=3 * KT) as w_pool, \
                 tc.tile_pool(name="stA", bufs=4) as stA, \
                 tc.tile_pool(name="psA", bufs=8, space="PSUM") as psA:

                HQ = [nc.sync, nc.scalar]
                # weights on SWDGE upfront; xt on the two HWDGE queues in
                # per-chunk [128, 512] tiles issued chunk-major so the first
                # q chunk's operands arrive first (the DMA fabric is a shared
                # serial resource -- JIT arrival order matters)
                w_ts = {}
                for wname, wdram in (("wq", wq), ("wk", wk)):
                    lst = []
                    for kt in range(KT):
                        t = w_pool.tile([P, CW], F32R, tag="w",
                                        name=f"{wname}_{kt}")
                        nc.gpsimd.dma_start(t, wdram[kt * P:(kt + 1) * P, :])
                        lst.append(t)
                    w_ts[wname] = lst
                    if wname == "wq":
                        nc.gpsimd.dma_start(bq_sb, bq[:])
                        nc.gpsimd.dma_start(bk_sb, bk[:])
                xt_t = [[None] * NQC for _ in range(KT)]

                def load_xt_chunk(c4):
                    for kt in range(KT):
                        t = xt_pool.tile([P, 512], F32R, tag="xt",
                                         name=f"xt{kt}_{c4}")
                        HQ[kt % 2].dma_start(
                            t, xt[kt * P:(kt + 1) * P, c4 * 512:(c4 + 1) * 512])
                        xt_t[kt][c4] = t

                load_xt_chunk(0)
                load_xt_chunk(1)
                # wv + bv on the scalar HWDGE queue: lands after the first two
                # xt chunks, before the first v sub-pass needs it (SWDGE
                # descriptor-gen would deliver it too late)
                lst = []
                for kt in range(KT):
                    t = w_pool.tile([P, CW], F32R, tag="w", name=f"wv_{kt}")
                    nc.scalar.dma_start(t, wv[kt * P:(kt + 1) * P, :])
                    lst.append(t)
                w_ts["wv"] = lst
                nc.scalar.dma_start(
                    bv_sb, bass.AP(tensor=bv, offset=0, ap=[[0, P], [1, CW]])
                )
                load_xt_chunk(2)
                load_xt_chunk(3)

                # PE warm-up: spin matmuls on the const tiles while the
                # first xt/wq DMAs are in flight -- keeps the HAM clock-gate
                # warm so the first real matmuls run at full rate
                ps_warm = psA.tile([P, 512], F32, tag="psA", name="ps_warm")
                for wi in range(14):
                    nc.tensor.matmul(ps_warm, ident_r, mask_r[:, 128:640],
                                     start=True, stop=True)

                def a_qk(wname, c4):
                    bias_sb = bq_sb if wname == "wq" else bk_sb
                    scratch = qT_d if wname == "wq" else None
                    w_t = w_ts[wname]
                    pss = [psA.tile([P, 512], F32, tag="psA",
                                    name=f"psA{c4}_{h}") for h in range(HH)]
                    for kt in range(KT):
                        for h in range(HH):
                            nc.tensor.matmul(
                                pss[h],
                                w_t[kt][:, h * HD:(h + 1) * HD],
                                xt_t[kt][c4],
                                start=(kt == 0), stop=(kt == KT - 1),
                            )
                    for h in range(HH):
                        if scratch is None:
                            # kT: bias-add straight into resident SBUF
                            nc.scalar.activation(
                                kT_all[:, h, c4 * 512:(c4 + 1) * 512],
                                pss[h], AF.Identity,
                                bias=bias_sb[:, h:h + 1], scale=1.0,
                            )
                        else:
                            st = stA.tile([P, 512], F32R, tag="stA",
                                          name="stA_qk")
                            nc.scalar.activation(
                                st, pss[h], AF.Identity,
                                bias=bias_sb[:, h:h + 1], scale=1.0,
                            )
                            nc.gpsimd.dma_start(
                                scratch[h][:, c4 * 512:(c4 + 1) * 512], st
                            )

                def a_v(c4):
                    w_t = w_ts["wv"]
                    for st16 in range(4 * c4, 4 * c4 + 4):
                        psv = psA.tile([P, 512], F32, tag="psA",
                                       name=f"psV{st16}")
                        for kt in range(KT):
                            nc.tensor.matmul(
                                psv,
                                xt_t[kt][c4][:, (st16 % 4) * P:
                                             (st16 % 4 + 1) * P],
                                w_t[kt],
                                start=(kt == 0), stop=(kt == KT - 1),
                            )
                        st = stA.tile([P, 512], F32R, tag="stA", name="stA_v")
                        nc.vector.tensor_tensor(st, psv, bv_sb, ALU.add)
                        HQ[st16 % 2].dma_start(
                            v_d[st16 * P:(st16 + 1) * P, :], st)

                # chunk-group order: xt chunk c4 dies after a_v(c4), so only
                # two chunks of xt are ever resident
                a_qk("wq", 0)
                a_qk("wq", 1)
                a_qk("wk", 0)
                a_v(0)
                a_qk("wq", 2)
                a_qk("wk", 1)
                a_v(1)
                a_qk("wq", 3)
                a_qk("wk", 2)
                a_v(2)
                a_qk("wk", 3)
                a_v(3)

            # ---------------- Phases B+C shared tiles ----------------
            with tc.tile_pool(name="bc_pool", bufs=1) as bc_pool:
                ctxT_sb = bc_pool.tile([P, HH, S], F32R)

                # ---------------- Phase B: attention ----------------
                with tc.tile_pool(name="qkv_pool", bufs=2) as qkv_pool, \
                     tc.tile_pool(name="vh_pool", bufs=2) as vh_pool, \
                     tc.tile_pool(name="pT_pool", bufs=2 * NKT + 12) as pT_pool, \
                     tc.tile_pool(name="accB", bufs=2) as accB, \
                     tc.tile_pool(name="stB", bufs=2) as stB, \
                     tc.tile_pool(name="psS", bufs=4, space="PSUM") as psS, \
                     tc.tile_pool(name="psCtx", bufs=3, space="PSUM") as psCtx, \
                     tc.tile_pool(name="psT", bufs=1, space="PSUM") as psT, \
                     tc.tile_pool(name="ppool", bufs=4) as ppool, \
                     nc.allow_low_precision(
                         reason="float32r tiles are 4-byte fp32 containers; "
                                "PE rounds on read, DVE writes full fp32 bits"):

                    def b_scores(h, j, qT_sb):
                        # scoresT blocks + exp for q chunk j; diagonal blocks
                        # get the causal mask added in PSUM by a 2nd matmul
                        nkt = 4 * j + 4
                        qs = qT_sb[:, j * 512:(j + 1) * 512]
                        pt_t = []
                        for i in range(nkt):
                            ps_s = psS.tile([P, 512], F32, tag="ps_s")
                            m = i - 4 * j
                            # diagonal blocks: only columns >= 128*m are live;
                            # strip the matmul when the narrower width still
                            # runs at 1 cyc/row (fp32r needs N >= 256)
                            c0 = P * m if m in (1, 2) else 0
                            nc.tensor.matmul(
                                ps_s[:, c0:],
                                kT_all[:, h, i * P:(i + 1) * P], qs[:, c0:],
                                start=True, stop=(m < 0),
                            )
                            if m >= 0:
                                nc.tensor.matmul(
                                    ps_s[:, c0:], ident_r,
                                    mask_r[:, 384 - P * m + c0:896 - P * m],
                                    start=False, stop=True,
                                )
                            pt = pT_pool.tile([P, 512], F32R, tag="pt",
                                              name=f"pt{h}_{j}_{i}")
                            if m > 0:
                                # columns < 128*m are fully masked: zero them
                                # on DVE and exp only the live strip (ACT is
                                # the phase-B pacer)
                                nc.vector.memset(
                                    pt.bitcast(F32)[:, :P * m], 0.0)
                                nc.scalar.activation(
                                    pt[:, P * m:], ps_s[:, P * m:],
                                    AF.Exp, scale=SCALE)
                            else:
                                nc.scalar.activation(pt, ps_s, AF.Exp,
                                                     scale=SCALE)
                            pt_t.append(pt)
                        return pt_t

                    def b_tail(h, j, v_t, pt_t):
                        # ctxT and denominator PSUM accumulations, then
                        # normalize into ctxT_sb
                        nkt = 4 * j + 4
                        ps_c = psCtx.tile([P, 512], F32, tag="ps_c")
                        for i in range(nkt):
                            m = i - 4 * j
                            c0 = P * m if m in (1, 2) else 0
                            nc.tensor.matmul(
                                ps_c[:, c0:], v_t[i], pt_t[i][:, c0:],
                                start=(i == 0), stop=(i == nkt - 1),
                            )
                        # pairwise pre-sums on DVE halve the denominator
                        # matmul count
                        npair = nkt // 2
                        psums = []
                        for i in range(npair):
                            pp = ppool.tile([P, 512], F32R, tag="ppair",
                                            name=f"pp{h}_{j}_{i}")
                            nc.vector.tensor_tensor(
                                pp, pt_t[2 * i], pt_t[2 * i + 1], ALU.add)
                            psums.append(pp)
                        ps_d = psT.tile([1, 512], F32, tag="ps_db", name="ps_d")
                        for i in range(npair):
                            nc.tensor.matmul(
                                ps_d, ones_col, psums[i],
                                start=(i == 0), stop=(i == npair - 1),
                            )
                        rden = accB.tile([1, 512], F32R, tag="rden")
                        nc.vector.reciprocal(rden, ps_d)
                        ps_b = psT.tile([P, 512], F32, tag="ps_db", name="ps_b")
                        nc.tensor.matmul(ps_b, ones_row, rden,
                                         start=True, stop=True)
                        rdenb = stB.tile([P, 512], F32, tag="rdenb")
                        nc.vector.tensor_copy(rdenb, ps_b)
                        nc.vector.tensor_tensor(
                            ctxT_sb[:, h, j * 512:(j + 1) * 512],
                            ps_c, rdenb, ALU.mult,
                        )

                    HQ = [nc.sync, nc.scalar]

                    def load_head(h):
                        qT_sb = qkv_pool.tile([P, S], F32R, tag="qT",
                                              name=f"qT{h}")
                        nc.sync.dma_start(qT_sb, qT_d[h])
                        # one rearranged 1MB DMA instead of 16 per-tile loads:
                        # HWDGE queue-processing time (~0.6us per dma) was
                        # stalling the seam, not bandwidth
                        v_all = vh_pool.tile([P, NKT, HD], F32R, tag="vh",
                                             name=f"vh{h}")
                        HQ[h % 2].dma_start(
                            v_all,
                            v_d[:, h * HD:(h + 1) * HD].rearrange(
                                "(i p) d -> p i d", p=P),
                        )
                        v_t = [v_all[:, i, :] for i in range(NKT)]
                        return qT_sb, v_t

                    # tails lag scores by two chunks: the ACT exp stream of
                    # chunk j must finish before tail(j)'s last ctx matmul,
                    # so give PE two chunks of score work to chew in between
                    from collections import deque
                    pend = deque()
                    loaded = load_head(0)
                    for h in range(HH):
                        qT_sb, v_t = loaded
                        if h + 1 < HH:
                            loaded = load_head(h + 1)
                        for j in range(NQC):
                            pt_t = b_scores(h, j, qT_sb)
                            pend.append((h, j, v_t, pt_t))
                            # at a head boundary the j=3 tail needs 16 ACT
                            # exps; delay it one extra score block so the PE
                            # has work while ACT drains
                            if j == NQC - 1 and h < HH - 1:
                                continue
                            if len(pend) > 1:
                                b_tail(*pend.popleft())
                    while pend:
                        b_tail(*pend.popleft())

                # ---------------- Phase C: output projection ----------------
                with tc.tile_pool(name="wp_pool", bufs=HH) as wp_pool, \
                     tc.tile_pool(name="outC", bufs=8) as outC, \
                     tc.tile_pool(name="psC", bufs=8, space="PSUM") as psC:
                    wp_t = []
                    for hh in range(HH):
                        t = wp_pool.tile([P, D], F32R, tag="wp", name=f"wp{hh}")
                        nc.gpsimd.dma_start(t, wp[hh * P:(hh + 1) * P, :])
                        wp_t.append(t)
                    for t16 in range(NKT):
                        for c4 in range(NQC):
                            ps_o = psC.tile([P, 512], F32, tag="psC",
                                            name=f"psC{t16}_{c4}")
                            for hh in range(HH):
                                nc.tensor.matmul(
                                    ps_o,
                                    ctxT_sb[:, hh, t16 * P:(t16 + 1) * P],
                                    wp_t[hh][:, c4 * 512:(c4 + 1) * 512],
                                    start=(hh == 0), stop=(hh == HH - 1),
                                )
                            o_st = outC.tile([P, 512], F32, tag="out",
                                             name=f"out{t16}_{c4}")
                            nc.any.tensor_copy(o_st, ps_o)
                            [nc.sync, nc.scalar][(t16 + c4) % 2].dma_start(
                                out[t16 * P:(t16 + 1) * P,
                                    c4 * 512:(c4 + 1) * 512], o_st)

    nc.compile()
    return nc


def _get_nc():
    if "nc" not in _cache:
        _cache["nc"] = _build()
    return _cache["nc"]


def _in_maps(x, wq, bq, wk, bk, wv, bv, wp):
    x = np.asarray(x, dtype=np.float32)
    maps = []
    xT = [np.ascontiguousarray(x[b].T) for b in range(B)]
    for c in range(NCORES):
        b = c // 4
        cols = slice((c % 4) * CW, (c % 4) * CW + CW)
        maps.append({
            "xt": xT[b],
            "wq": np.ascontiguousarray(np.asarray(wq, np.float32)[:, cols]),
            "wk": np.ascontiguousarray(np.asarray(wk, np.float32)[:, cols]),
            "wv": np.ascontiguousarray(np.asarray(wv, np.float32)[:, cols]),
            "wp": np.ascontiguousarray(np.asarray(wp, np.float32)[cols, :]),
            "bq": np.ascontiguousarray(
                np.asarray(bq, np.float32)[cols].reshape(HH, P).T),
            "bk": np.ascontiguousarray(
                np.asarray(bk, np.float32)[cols].reshape(HH, P).T),
            "bv": np.ascontiguousarray(np.asarray(bv, np.float32)[cols]),
        })
    return maps


def kernel(x, wq, bq, wk, bk, wv, bv, wp, bp):
    from concourse.bass_utils import run_bass_kernel_spmd

    nc = _get_nc()
    maps = _in_maps(x, wq, bq, wk, bk, wv, bv, wp)
    res = run_bass_kernel_spmd(nc, maps, core_ids=list(range(NCORES)))
    parts = [res.results[c]["out"] for c in range(NCORES)]
    bp = np.asarray(bp, dtype=np.float32)
    full = np.empty((B, S, D), dtype=np.float32)
    for b in range(B):
        acc = parts[4 * b].astype(np.float64)
        for c in range(4 * b + 1, 4 * b + 4):
            acc += parts[c]
        full[b] = (acc + bp).astype(np.float32)
    return full



# revision 24
# speedup vs baseline: 1.2920x; 1.2920x over previous
"""Multi-head causal attention (B=2, S=2048, D=2048, H=16) on 8 TRN2 NeuronCores.

Sharding (host-side): core c handles batch b=c//4 and heads 4*(c%4)..+4
(512-wide column slice of wq/wk/wv, row slice of wp).  Each core computes its
4 heads' attention and a partial output projection [S, D] in bf16; the host
sums the 4 partials per batch, rescales, and adds bp.

Per-core kernel — precision strategy (rel err ~4e-3, gate is 2e-2):
  * QKV and output projections run as fp8e4m3 DoubleRow matmuls (0.5 cyc/row,
    K=256 per instruction) with RESIDUAL operands: A ~= A_hi + A_lo where both
    are fp8 and A_lo = Q8(A - A_hi).  Three DR matmuls (hi*hi, lo*hi, hi*lo)
    replace two fp32r matmuls -> 0.375 cyc/row-of-128K, ~1e-3 error.
    x/w splits are prepared on the host; ctx hi/lo are built on-chip.
  * scores / softmax / ctx run in bf16 (same 1 cyc/row as fp32r, half SBUF,
    DVE 2x mode).  Causal mask is added in PSUM by one [128,128] triangle
    matmul per diagonal block (the mask only acts on a 128-wide strip).
  * softmax denominators: bf16 pairwise tree on DVE, then ONE gpsimd
    partition_all_reduce per (h,j) -> broadcast f32 row sums with zero PE
    cycles; reciprocal on DVE; normalize+scale in one scalar_tensor_tensor.
  * Schedule: phase A chunk c4 feeds attention chunk j=c4-1, so A's PE-dense
    projection slots interleave with B's ACT-bound score/exp stream at
    2-score-tile granularity; output-projection tiles (phase C) drain into
    every remaining PE gap.  Output tiles stored bf16 (host rescales).
"""
import sys
if "/opt/trn_rl_repo" not in sys.path:
    sys.path.insert(0, "/opt/trn_rl_repo")

import numpy as np
import ml_dtypes

import os
B, S, D = 2, 2048, 2048
PSA = int(os.environ.get("K_PSA", "3"))
PSS = int(os.environ.get("K_PSS", "3"))
WARM = int(os.environ.get("K_WARM", "22"))
ADV = int(os.environ.get("K_ADV", "2"))
INUNIT = int(os.environ.get("K_INUNIT", "1"))
H, HD = 16, 128
NCORES = 8
HH = 4            # heads per core
CW = HH * HD      # 512 column slice per core
P = 128
KT = D // P       # 16 contraction tiles
NQC = 4           # q chunks of 512
NKT = S // P      # 16 key tiles
SCALE = 1.0 / float(np.sqrt(HD))
MASK_NEG = -1.0e10

S_W = 32.0        # weight quant scale (wq/wk/wv)
S_C = 16.0        # ctx quant scale
S_WP = 32.0       # wp quant scale
OUT_DIV = S_C * S_WP

F8NP = ml_dtypes.float8_e4m3

_cache = {}


def _build():
    import concourse.bass as bass
    import concourse.bass_isa as bass_isa
    import concourse.tile as tile
    from concourse import bacc, mybir
    from collections import deque

    F32 = mybir.dt.float32
    BF16 = mybir.dt.bfloat16
    F8 = mybir.dt.float8e4
    AF = mybir.ActivationFunctionType
    ALU = mybir.AluOpType
    DR = mybir.MatmulPerfMode.DoubleRow

    nc = bacc.Bacc("TRN2", target_bir_lowering=False, debug=False, num_devices=NCORES)

    # host-tiled fp8 inputs: x (hi/lo) as [chunk, p, kt, 512] so each chunk is
    # one linear DMA; weights as [p, kt, 512]; wp as [p, pair, slot, D]
    xh = nc.dram_tensor("xh", [NQC, P, KT, 512], F8, kind="ExternalInput")
    xl = nc.dram_tensor("xl", [NQC, P, KT, 512], F8, kind="ExternalInput")
    w_names = ["wqh", "wql", "wkh", "wkl", "wvh", "wvl"]
    w_dram = {n: nc.dram_tensor(n, [P, KT, 512], F8, kind="ExternalInput")
              for n in w_names}
    wph = nc.dram_tensor("wph", [P, 2, 2, D], F8, kind="ExternalInput")
    wpl = nc.dram_tensor("wpl", [P, 2, 2, D], F8, kind="ExternalInput")
    bq = nc.dram_tensor("bq", [P, HH], F32, kind="ExternalInput")
    bk = nc.dram_tensor("bk", [P, HH], F32, kind="ExternalInput")
    bv = nc.dram_tensor("bv", [CW], F32, kind="ExternalInput")
    out = nc.dram_tensor("out", [S, D], BF16, kind="ExternalOutput")

    with tile.TileContext(nc) as tc:
        with tc.tile_pool(name="consts", bufs=1) as consts, \
             tc.tile_pool(name="qT_pool", bufs=2) as qT_pool, \
             tc.tile_pool(name="c_pool", bufs=8) as c_pool, \
             tc.tile_pool(name="wp_pool", bufs=2) as wp_pool, \
             tc.tile_pool(name="pt_pool", bufs=20) as pt_pool, \
             tc.tile_pool(name="tree", bufs=4) as tree_pool, \
             tc.tile_pool(name="tmp_pool", bufs=2) as tmp_pool, \
             tc.tile_pool(name="den_pool", bufs=2) as den_pool, \
             tc.tile_pool(name="o_pool", bufs=3) as o_pool, \
             tc.tile_pool(name="psS", bufs=PSS, space="PSUM") as psS, \
             tc.tile_pool(name="psCtx", bufs=2, space="PSUM") as psCtx, \
             nc.allow_low_precision(
                 reason="bf16/fp8 stores are deliberate: residual-fp8 matmuls "
                        "and bf16 softmax keep rel err ~4e-3 vs the 2e-2 gate"):
            # causal-triangle mask [128,128] (0 / -1e10), identity, both bf16
            scratch_f = consts.tile([P, P], F32)
            ident_bf = consts.tile([P, P], BF16)
            mask_bf = consts.tile([P, P], BF16)
            nc.vector.memset(scratch_f, 0.0)
            nc.gpsimd.affine_select(
                out=scratch_f, in_=scratch_f,
                compare_op=ALU.not_equal, fill=1.0,
                base=0, channel_multiplier=1, pattern=[[-1, P]],
            )
            nc.vector.tensor_copy(ident_bf, scratch_f)
            nc.vector.memset(scratch_f, 0.0)
            # mask(p, c) = -1e10 where p > c (affine_select keeps the input
            # where the predicate c - p >= 0 holds, fills elsewhere)
            nc.gpsimd.affine_select(
                out=scratch_f, in_=scratch_f,
                compare_op=ALU.is_ge, fill=MASK_NEG,
                base=0, channel_multiplier=-1, pattern=[[1, P]],
            )
            nc.vector.tensor_copy(mask_bf, scratch_f)
            warm_bf = consts.tile([P, 512], BF16)
            nc.vector.memset(warm_bf.bitcast(F32), 0.0)

            bq_sb = consts.tile([P, HH], F32)
            bk_sb = consts.tile([P, HH], F32)
            bv_sb = consts.tile([P, CW], F32)

            kT_all = consts.tile([P, HH, S], BF16)
            v_all = consts.tile([P, NKT, CW], BF16)
            qT_c = [None] * NQC          # per-chunk [P, HH, 512] bf16
            chi_c = [None] * NQC         # per-chunk [P, HH, 512] fp8
            clo_c = [None] * NQC

            wph_t = wp_pool.tile([P, 2, 2, D], F8, tag="wp", name="wph")
            wpl_t = wp_pool.tile([P, 2, 2, D], F8, tag="wp", name="wpl")

            pend_C = deque()
            o_late = {"pool": None}

            def c_tile(j, t16, c4):
                """one output-projection tile [128 q, 512 dcols]."""
                r = 4 * j + t16
                ps = o_late["pool"].tile([P, 512], F32, tag="ps_o")
                idx = 0
                for pr in range(2):
                    for lh, rh in ((chi_c[j], wph_t), (clo_c[j], wph_t),
                                   (chi_c[j], wpl_t)):
                        nc.tensor.matmul(
                            ps,
                            lh[:, 2 * pr:2 * pr + 2, t16 * P:(t16 + 1) * P],
                            rh[:, pr, :, c4 * 512:(c4 + 1) * 512],
                            start=(idx == 0), stop=(idx == 5),
                            perf_mode=DR,
                        )
                        idx += 1
                o = o_pool.tile([P, 512], BF16, tag="o", name=f"o{r}_{c4}")
                if (r + c4) % 2:
                    nc.scalar.activation(o, ps, AF.Copy)
                else:
                    nc.vector.tensor_copy(o, ps)
                [nc.sync, nc.scalar][(r + c4) % 2].dma_start(
                    out[r * P:(r + 1) * P, c4 * 512:(c4 + 1) * 512], o)

            def drain_C(n):
                for _ in range(n):
                    if not pend_C:
                        return
                    pend_C.popleft()()

            def ctx_mm(ps_c, h, j, i, pt, nkt):
                m = i - 4 * j
                c0 = P * m if m in (1, 2) else 0
                nc.tensor.matmul(
                    ps_c[:, c0:], v_all[:, i, h * HD:(h + 1) * HD],
                    pt[:, c0:],
                    start=(i == 0), stop=(i == nkt - 1),
                )

            def s_unit(h, j):
                """One (head, q-chunk): scores + mask + exp, with the ctx
                matmuls and the DVE denominator chain pipelined one tile
                behind, then the finisher (reduce/normalize/quantize).
                Yields after each score tile so other PE work interleaves."""
                nkt = 4 * j + 4
                pt_t = []
                ps_c = psCtx.tile([P, 512], F32, tag="ps_c")
                acc = None
                for i in range(nkt):
                    m = i - 4 * j
                    c0 = P * m if m > 0 else 0
                    ps = psS.tile([P, 512], F32, tag="ps_s")
                    nc.tensor.matmul(
                        ps[:, c0:],
                        kT_all[:, h, i * P:(i + 1) * P],
                        qT_c[j][:, h, c0:],
                        start=True, stop=(m < 0),
                    )
                    if m >= 0:
                        nc.tensor.matmul(
                            ps[:, P * m:P * m + P], ident_bf, mask_bf,
                            start=False, stop=True,
                        )
                    pt = pt_pool.tile([P, 512], BF16, tag="pt",
                                      name=f"pt{h}_{j}_{i}")
                    if m > 0:
                        nc.vector.memset(pt.bitcast(F32)[:, :64 * m], 0.0)
                    nc.scalar.activation(pt[:, c0:], ps[:, c0:],
                                         AF.Exp, scale=SCALE)
                    pt_t.append(pt)
                    # one-tile-lagged pipeline: ctx matmul + chain add for i-1
                    if INUNIT and i >= 1:
                        ctx_mm(ps_c, h, j, i - 1, pt_t[i - 1], nkt)
                    if INUNIT and i == 2:
                        acc = tree_pool.tile([P, 512], BF16, tag="tr",
                                             name=f"t{h}{j}a")
                        nc.vector.tensor_tensor(acc, pt_t[0], pt_t[1], ALU.add)
                    elif INUNIT and i >= 3:
                        nxt = tree_pool.tile([P, 512], BF16, tag="tr",
                                             name=f"t{h}{j}{i}")
                        nc.vector.tensor_tensor(nxt, acc, pt_t[i - 1], ALU.add)
                        acc = nxt
                    yield
                yield ("done", h, j, pt_t, ps_c, acc)

            def t_unit(h, j, pt_t, ps_c, acc):
                """finisher: last ctx matmul + denominator + normalize."""
                nkt = 4 * j + 4
                lo = 1 if INUNIT else 0
                if not INUNIT:
                    for i in range(nkt - 1):
                        ctx_mm(ps_c, h, j, i, pt_t[i], nkt)
                    acc = tree_pool.tile([P, 512], BF16, tag="tr",
                                         name=f"tc{h}{j}")
                    nc.vector.tensor_tensor(acc, pt_t[0], pt_t[1], ALU.add)
                    for i in range(2, nkt - 1):
                        n2 = tree_pool.tile([P, 512], BF16, tag="tr",
                                            name=f"tc{h}{j}{i}")
                        nc.vector.tensor_tensor(n2, acc, pt_t[i], ALU.add)
                        acc = n2
                ctx_mm(ps_c, h, j, nkt - 1, pt_t[nkt - 1], nkt)
                nxt = tree_pool.tile([P, 512], BF16, tag="tr", name=f"tf{h}{j}")
                nc.vector.tensor_tensor(nxt, acc, pt_t[nkt - 1], ALU.add)
                den = den_pool.tile([P, 512], F32, tag="den", name=f"dn{h}{j}")
                nc.gpsimd.partition_all_reduce(
                    den, nxt, channels=P, reduce_op=bass_isa.ReduceOp.add)
                rden = den_pool.tile([P, 512], F32, tag="den", name=f"rd{h}{j}")
                nc.vector.reciprocal(rden, den)
                tmp = tmp_pool.tile([P, 512], F32, tag="tmp", name=f"tm{h}{j}")
                nc.vector.scalar_tensor_tensor(
                    out=tmp, in0=ps_c, scalar=S_C, in1=rden,
                    op0=ALU.mult, op1=ALU.mult)
                if h == 0:
                    chi_c[j] = c_pool.tile([P, HH, 512], F8, tag="chi",
                                           name=f"chi{j}")
                    clo_c[j] = c_pool.tile([P, HH, 512], F8, tag="clo",
                                           name=f"clo{j}")
                if j == NQC - 1 and h == HH - 1:
                    nc.vector.tensor_copy(chi_c[j][:, h, :], tmp)
                else:
                    nc.gpsimd.tensor_copy(chi_c[j][:, h, :], tmp)
                nc.vector.scalar_tensor_tensor(
                    out=clo_c[j][:, h, :], in0=tmp, scalar=1.0,
                    in1=chi_c[j][:, h, :], op0=ALU.mult, op1=ALU.subtract)

            def finish_unit(token):
                _, h, j, pt_t, ps_c, acc = token
                t_unit(h, j, pt_t, ps_c, acc)
                if h == HH - 1:
                    for t16 in range(4):
                        for c4 in range(NQC):
                            pend_C.append(
                                (lambda jj=j, tt=t16, cc=c4:
                                 c_tile(jj, tt, cc)))

            def advance_b(gens, nscores):
                """advance the B score stream by nscores tiles."""
                done = 0
                while gens and done < nscores:
                    try:
                        tok = next(gens[0][1])
                    except StopIteration:
                        gens.popleft()
                        continue
                    if tok is None:
                        done += 1
                    else:
                        finish_unit(tok)
                return done

            def drain_b_upto(gens, jmax):
                """fully drain B units with j <= jmax (pool-reuse guard)."""
                while gens and gens[0][0] <= jmax:
                    try:
                        tok = next(gens[0][1])
                    except StopIteration:
                        gens.popleft()
                        continue
                    if tok is not None:
                        finish_unit(tok)

            # ---------------- interleaved emission ----------------
            with tc.tile_pool(name="x_pool", bufs=4) as x_pool, \
                 tc.tile_pool(name="w_pool", bufs=6) as w_pool, \
                 tc.tile_pool(name="psA", bufs=PSA, space="PSUM") as psA:

                w_t = {}
                x_t = [None] * NQC

                def load_x(c4):
                    th = x_pool.tile([P, KT, 512], F8, tag="x", name=f"xh{c4}")
                    tl = x_pool.tile([P, KT, 512], F8, tag="x", name=f"xl{c4}")
                    nc.sync.dma_start(th, xh[c4])
                    nc.scalar.dma_start(tl, xl[c4])
                    x_t[c4] = (th, tl)

                # DMA-engine arrival order matters (transfers serialize):
                # xh0 (sync) || [wqh, wql, bq, bk, xl0] (scalar); everything
                # else on SWDGE; next x chunk deferred so it can't jump ahead
                th0 = x_pool.tile([P, KT, 512], F8, tag="x", name="xh0")
                tl0 = x_pool.tile([P, KT, 512], F8, tag="x", name="xl0")
                nc.sync.dma_start(th0, xh[0])
                for n in ("wqh", "wql"):
                    t = w_pool.tile([P, KT, 512], F8, tag="w", name=n)
                    nc.scalar.dma_start(t, w_dram[n][:])
                    w_t[n] = t
                nc.scalar.dma_start(bq_sb, bq[:])
                nc.scalar.dma_start(bk_sb, bk[:])
                nc.scalar.dma_start(tl0, xl[0])
                x_t[0] = (th0, tl0)
                for n in ("wkh", "wkl", "wvh", "wvl"):
                    t = w_pool.tile([P, KT, 512], F8, tag="w", name=n)
                    nc.gpsimd.dma_start(t, w_dram[n][:])
                    w_t[n] = t
                nc.gpsimd.dma_start(
                    bv_sb, bass.AP(tensor=bv, offset=0, ap=[[0, P], [1, CW]]))
                nc.gpsimd.dma_start(wph_t, wph[:])
                nc.gpsimd.dma_start(wpl_t, wpl[:])

                # PE warm-up while the first DMAs land
                ps_w = psA.tile([P, 512], F32, tag="psA", name="warm")
                for _ in range(WARM):
                    nc.tensor.matmul(ps_w, ident_bf, warm_bf,
                                     start=True, stop=True)

                def a_qk_slot(wn, c4, h):
                    whn, wln = (("wqh", "wql") if wn == "q" else ("wkh", "wkl"))
                    bias = bq_sb if wn == "q" else bk_sb
                    xhi, xlo = x_t[c4]
                    ps = psA.tile([P, 512], F32, tag="psA", name=f"p{wn}{c4}{h}")
                    first = True
                    mcol = slice(h * HD, (h + 1) * HD)
                    for term, (wt, xt) in enumerate((
                            (w_t[whn], xhi), (w_t[wln], xhi), (w_t[whn], xlo))):
                        for t in range(KT // 2):
                            nc.tensor.matmul(
                                ps, wt[:, 2 * t:2 * t + 2, mcol],
                                xt[:, 2 * t:2 * t + 2, :],
                                start=first, stop=(term == 2 and t == KT // 2 - 1),
                                perf_mode=DR)
                            first = False
                    dst = (qT_c[c4] if wn == "q" else kT_all)
                    dslice = (dst[:, h, :] if wn == "q"
                              else dst[:, h, c4 * 512:(c4 + 1) * 512])
                    nc.scalar.activation(dslice, ps, AF.Identity,
                                         bias=bias[:, h:h + 1], scale=1.0 / S_W)

                def a_v_slot(c4, qs):
                    st16 = 4 * c4 + qs
                    xhi, xlo = x_t[c4]
                    ps = psA.tile([P, 512], F32, tag="psA", name=f"pv{st16}")
                    first = True
                    for term, (lt, rt) in enumerate((
                            (xhi, w_t["wvh"]), (xhi, w_t["wvl"]),
                            (xlo, w_t["wvh"]))):
                        for t in range(KT // 2):
                            nc.tensor.matmul(
                                ps, lt[:, 2 * t:2 * t + 2, qs * P:(qs + 1) * P],
                                rt[:, 2 * t:2 * t + 2, :],
                                start=first, stop=(term == 2 and t == KT // 2 - 1),
                                perf_mode=DR)
                            first = False
                    nc.vector.scalar_tensor_tensor(
                        out=v_all[:, st16, :], in0=ps, scalar=1.0 / S_W,
                        in1=bv_sb, op0=ALU.mult, op1=ALU.add)

                bgens = deque()
                for c4 in range(NQC):
                    qT_c[c4] = qT_pool.tile([P, HH, 512], BF16, tag="qT",
                                            name=f"qT{c4}")
                    if c4 >= 1:
                        for h in range(HH):
                            bgens.append((c4 - 1, s_unit(h, c4 - 1)))
                    slots = ([("q", c4, h) for h in range(HH)]
                             + [("k", c4, h) for h in range(HH)]
                             + [("v", c4, qs) for qs in range(4)])
                    for si, (kind, cc, idx) in enumerate(slots):
                        if kind == "q":
                            a_qk_slot("q", cc, idx)
                        elif kind == "k":
                            a_qk_slot("k", cc, idx)
                        else:
                            a_v_slot(cc, idx)
                        if si == 6 and c4 + 1 < NQC:
                            load_x(c4 + 1)
                        advance_b(bgens, ADV)
                    # pool-reuse guard: qT slot c4+1 (bufs=2) needs B(c4-1)
                    # finished before the next chunk allocates it
                    if c4 + 1 < NQC:
                        drain_b_upto(bgens, c4 - 1)

            # x/w/psA freed; open a second psO pool for the remaining C tiles
            with tc.tile_pool(name="psO2", bufs=3, space="PSUM") as psO2:
                o_late["pool"] = psO2
                for h in range(HH):
                    bgens.append((NQC - 1, s_unit(h, NQC - 1)))
                while bgens and advance_b(bgens, 2):
                    drain_C(1)
                drain_C(len(pend_C) + 1)

    nc.compile()
    return nc


def _get_nc():
    if "nc" not in _cache:
        _cache["nc"] = _build()
    return _cache["nc"]


def _split8(a, s):
    a = np.asarray(a, np.float32) * np.float32(s)
    hi = a.astype(F8NP)
    lo = (a - hi.astype(np.float32)).astype(F8NP)
    return hi, lo


def _tile_x(xt8):
    """[D, S] -> [NQC, 128, KT, 512] (chunk-major tiles)."""
    t = xt8.reshape(KT, P, NQC, 512)          # (kt, p, c4, s)
    return np.ascontiguousarray(t.transpose(2, 1, 0, 3))


def _tile_w(w8):
    """[D, CW] -> [128, KT, CW]."""
    return np.ascontiguousarray(w8.reshape(KT, P, CW).transpose(1, 0, 2))


def _in_maps(x, wq, bq, wk, bk, wv, bv, wp):
    x = np.asarray(x, dtype=np.float32)
    maps = []
    xparts = []
    for b in range(B):
        xh8, xl8 = _split8(np.ascontiguousarray(x[b].T), 1.0)
        xparts.append((_tile_x(xh8), _tile_x(xl8)))
    for c in range(NCORES):
        b = c // 4
        cols = slice((c % 4) * CW, (c % 4) * CW + CW)
        m = {"xh": xparts[b][0], "xl": xparts[b][1]}
        for name, w in (("wq", wq), ("wk", wk), ("wv", wv)):
            hi, lo = _split8(np.asarray(w, np.float32)[:, cols], S_W)
            m[name + "h"] = _tile_w(hi)
            m[name + "l"] = _tile_w(lo)
        wp_s = np.asarray(wp, np.float32)[cols, :]
        hi, lo = _split8(wp_s, S_WP)
        # [CW, D] -> [128, pr, slot, D]: row (2*pr+slot)*128 + p
        m["wph"] = np.ascontiguousarray(
            hi.reshape(2, 2, P, D).transpose(2, 0, 1, 3))
        m["wpl"] = np.ascontiguousarray(
            lo.reshape(2, 2, P, D).transpose(2, 0, 1, 3))
        m["bq"] = np.ascontiguousarray(
            np.asarray(bq, np.float32)[cols].reshape(HH, P).T)
        m["bk"] = np.ascontiguousarray(
            np.asarray(bk, np.float32)[cols].reshape(HH, P).T)
        m["bv"] = np.ascontiguousarray(np.asarray(bv, np.float32)[cols])
        maps.append(m)
    return maps


def kernel(x, wq, bq, wk, bk, wv, bv, wp, bp):
    from concourse.bass_utils import run_bass_kernel_spmd

    nc = _get_nc()
    maps = _in_maps(x, wq, bq, wk, bk, wv, bv, wp)
    res = run_bass_kernel_spmd(nc, maps, core_ids=list(range(NCORES)))
    bp = np.asarray(bp, dtype=np.float32)
    full = np.empty((B, S, D), dtype=np.float32)
    for b in range(B):
        acc = res.results[4 * b]["out"].astype(np.float64)
        for c in range(4 * b + 1, 4 * b + 4):
            acc += res.results[c]["out"].astype(np.float64)
        full[b] = (acc / OUT_DIV + bp).astype(np.float32)
    return full
